# revision 31
# baseline (speedup 1.0000x reference)
"""Trainium2 Bass kernel for nn_AttentionScore (sparse local attention scores).

Reference computation (B=4, C=64, N=16384, S=16):
    tmp   = xyz[:, :, :, None] - neighbor_xyz            # [B,3,N,S]
    pos   = concat([tmp, ||tmp||], axis=1)               # [B,4,N,S]
    k     = Wk @ (neighbor_points + Wpos @ pos + bpos)   # [B,C,N,S]
    attn  = softmax_s((points*scale) . k)                # [B,N,S]

Softmax over s is shift-invariant, so every term constant in s drops out:
    attn[m,s] ~ sum_c qW[c,m]*np[c,m,s] + sum_j qp[j,m]*tmp[j,m,s] + qp3[m]*||tmp||
with qW = (scale*Wk)^T @ points, qp = Wpos^T @ qW (bpos and the xyz.qp dot cancel).

Sharding: N split contiguously across 8 cores, m = b*2048 + n_local in
[0, 8192) per core. All bulk data ships as fp16 (gate is rel_err < 2e-2,
measured ~5e-4), halving HBM traffic.

v3 dataflow per core (PE matmul count halved via a channel fold-by-2):
  - NP staged [128 part = (g4, c_hi32), free = (i8, c_lo2, s16, mm256)] fp16
    (c = c_lo*32 + c_hi, m = g*2048 + i*256 + mm); 8 supertiles of 2 MB
    alternate between the two hardware DMA queues (16 KB descriptors).
  - DVE multiplies each c_lo half by qW (broadcast over s); the c_lo fold-add
    runs on DVE or the otherwise-idle Pool engine (split tuned for cadence).
  - TensorE reduces the 128 (g, c_hi) partitions per 32-m column block with a
    sliding block-ones lhsT from a selector ribbon; one matmul covers 4
    m-groups (one per g), so only 64 main matmuls. Rows land at p = m>>6 of
    two PSUM banks (bank = tt parity, cols = s*32 + m%32) - softmax layout.
  - qW reaches the (g, c_hi) layout via 16 small SBUF->SBUF relayout DMAs on
    the hardware queues (two waves, pipelined with phase 1).
  - attn2 (positional term, computed by DVE/Act in the same layout) is added
    into PSUM by identity-lhsT matmuls that close each bank's group.
  - Softmax epilogue skips the max-subtraction (|logit| < ~3, fp32 exp safe).
Phase 1 computes qW with 16 fp16 matmuls whose lhsT's unused half carries
(scale*Wk@Wpos), so qp rides along for free.
"""

import sys

sys.path.insert(0, "/opt/trn_rl_repo")

import numpy as np

import concourse.bass as bass
import concourse.bacc as bacc
import concourse.tile as tile
from concourse import mybir
from concourse.bass_utils import run_bass_kernel_spmd

F32 = mybir.dt.float32
F16 = mybir.dt.float16
AF = mybir.ActivationFunctionType
AX = mybir.AxisListType

B, C, N, S = 4, 64, 16384, 16
NCORES = 8
NL = N // NCORES            # 2048 points per core
M = B * NL                  # 8192 (b, n) rows per core
HALF = M // 2               # 4096 per h-half
MG = M // 4                 # 2048 m per g-group
NSUP = 8                    # supertiles per core
MSUP = MG // NSUP           # 256 m per g per supertile
SCALE = float(C) ** -0.5

# supertiles whose c_lo fold-add runs on the Pool engine instead of DVE.
# Pool TensorTensor measured ~2.4 ns/elem on HW (4x slower than DVE) - keep
# it off the supertile critical path; it only gets latency-insensitive work.
POOL_ADDS = (0, 1)  # pool is idle early; DVE keeps the steady-state adds
# which of the phase-2 ops run on Pool ("p") vs DVE ("d")
PH2_ENG = {"tmp": "p", "n2a": "p", "n2": "p", "u1": "p", "u": "p",
           "umul": "p", "a2": "p", "attn2": "p"}


def _body(tc):
    nc = tc.nc

    NPD = nc.dram_tensor("NP", [128, NSUP * 2 * S * MSUP], F16, kind="ExternalInput").ap()
    NXD = nc.dram_tensor("NX", [128, 3 * 2 * S * 32], F16, kind="ExternalInput").ap()
    XYZD = nc.dram_tensor("XYZ", [128, 3 * 2 * 32], F16, kind="ExternalInput").ap()
    PD = nc.dram_tensor("P", [C, M], F16, kind="ExternalInput").ap()
    WKHD = nc.dram_tensor("WKH", [C, 256], F16, kind="ExternalInput").ap()
    SELD = nc.dram_tensor("SEL", [128, 160], F16, kind="ExternalInput").ap()
    IDND = nc.dram_tensor("IDN", [128, 128], F16, kind="ExternalInput").ap()
    OUTD = nc.dram_tensor("OUT", [128, 1024], F32, kind="ExternalOutput").ap()

    with (
        tc.tile_pool(name="const", bufs=1) as cp,
        tc.tile_pool(name="ph2", bufs=1) as p2,
        tc.tile_pool(name="sm", bufs=1) as sp,
        tc.tile_pool(name="npt", bufs=4) as npp,
        tc.tile_pool(name="prod", bufs=2) as prp,
        tc.tile_pool(name="psq", bufs=4, space="PSUM") as psq,
        tc.tile_pool(name="psm", bufs=2, space="PSUM") as psm,
    ):
        # ---- input loads (queue order matters: queues process in order) ----
        wkh = cp.tile([C, 256], F16)
        nc.sync.dma_start(wkh[:], WKHD)
        p_sb = cp.tile([C, M], F16)
        nc.sync.dma_start(p_sb[:, 0:HALF], PD[:, 0:HALF])
        nxt = cp.tile([128, 3 * 2 * S * 32], F16)
        nc.scalar.dma_start(p_sb[:, HALF:M], PD[:, HALF:M])
        sel = cp.tile([128, 160], F16)
        nc.sync.dma_start(sel[:], SELD)
        nc.scalar.dma_start(nxt[:], NXD)  # after P-h1: phase 1 starts sooner
        xyzt = cp.tile([128, 3 * 2 * 32], F16)
        nc.scalar.dma_start(xyzt[:], XYZD)
        idn = cp.tile([128, 128], F16)
        nc.gpsimd.dma_start(idn[:], IDND)

        qw = cp.tile([128, HALF], F16)    # (h, c)-rows, col u: qW[c, h*HALF+u]
        qwG = cp.tile([128, 2 * MG], F16)  # (g, c_hi)-rows, free (c_lo2, u2048)
        qps = cp.tile([128, HALF], F16)   # rows 64:68 (h=0) / 0:4 (h=1): qp
        qptB = cp.tile([128, 4 * 2 * 32], F16)  # p-major: [p, (j4, b2, mm32)]
        attn2 = cp.tile([128, 1024], F16)       # [p, (b2, s16, mm32)]

        # NP supertile DMAs: emitted interleaved with the relayout waves so
        # each HW queue processes [NP tiles | ready scatters] in a stall-free
        # order (queues are strictly in-order).
        FS = 2 * S * MSUP  # 8192 free per supertile
        HS = S * MSUP      # 4096 per c_lo half
        npts = [npp.tile([128, FS], F16, name=f"npt{i}", tag="npt") for i in range(NSUP)]

        def np_dma(i):
            eng = nc.sync if i % 2 == 0 else nc.scalar
            eng.dma_start(npts[i][:], NPD[:, i * FS:(i + 1) * FS])

        np_dma(0)
        np_dma(1)

        def ve(which):
            return nc.gpsimd if PH2_ENG[which] == "p" else nc.vector

        # phase 2 head (pool): tmp = xyz - nx, runs in Pool's early idle window
        nx4 = nxt[:].rearrange("p (jb s mm) -> p jb s mm", jb=6, s=S, mm=32)
        xyz4 = (
            xyzt[:]
            .rearrange("p (jb one mm) -> p jb one mm", jb=6, one=1, mm=32)
            .broadcast_to((128, 6, S, 32))
        )
        tmp = p2.tile([128, 3072], F16)
        tmp4 = tmp[:].rearrange("p (jb s mm) -> p jb s mm", jb=6, s=S, mm=32)
        ve("tmp").tensor_sub(tmp4, xyz4, nx4)

        # ---- phase 1: qW (+ qp in spare lhsT columns) ----
        # chunk order interleaves the four g-groups so relayout wave q
        # (u-quarter q, every g) is ready after chunk 4q+3.
        order = [0, 4, 8, 12, 1, 5, 9, 13, 2, 6, 10, 14, 3, 7, 11, 15]
        pending = []
        for k, bm in enumerate(order):
            h = bm >> 3
            cc = bm & 7
            pq = psq.tile([128, 512], F32, name="pq", tag="pq")
            nc.tensor.matmul(
                pq[:],
                lhsT=wkh[:, h * 128:(h + 1) * 128],
                rhs=p_sb[:, bm * 512:(bm + 1) * 512],
                start=True,
                stop=True,
            )
            rows = slice(h * 64, h * 64 + 64)
            nc.scalar.copy(qw[rows, cc * 512:(cc + 1) * 512], pq[rows, :])
            qrows = slice(64, 68) if h == 0 else slice(0, 4)
            # defer the qp copies so Act's qw copies (which gate the relayout
            # waves) run back-to-back; psq bufs=4 keeps the pq tiles alive
            pending.append((qps[qrows, cc * 512:(cc + 1) * 512], pq[qrows, :]))

            if k % 4 == 3:
                # qw relayout wave q: (h,c)-rows -> (g,c_hi)-rows, u-quarter q.
                # waves 0,2 -> sync queue; 1,3 -> scalar queue.
                q = k // 4
                eng = nc.sync if q % 2 == 0 else nc.scalar
                for gh in range(2):
                    for gl in range(2):
                        for cl in range(2):
                            us = gl * 2048 + q * 512
                            ud = cl * 2048 + q * 512
                            src = qw[gh * 64 + cl * 32: gh * 64 + cl * 32 + 32, us:us + 512]
                            dst = qwG[gh * 64 + gl * 32: gh * 64 + gl * 32 + 32, ud:ud + 512]
                            eng.dma_start(dst, src)
                for dst, src in pending:
                    nc.scalar.copy(dst, src)
                pending = []
                if q == 0:
                    np_dma(2)
                elif q == 1:
                    np_dma(3)
                elif q == 2:
                    np_dma(4)
                    np_dma(5)

        np_dma(6)
        # qp scatter into p-major layout:
        # qptB[p, j*64+b*32+mm] = qp[j, p*64+b*32+mm]
        for j in range(4):
            nc.scalar.dma_start(qptB[0:64, j * 64:(j + 1) * 64], qps[64 + j:65 + j, :])
            nc.scalar.dma_start(qptB[64:128, j * 64:(j + 1) * 64], qps[j:j + 1, :])
        np_dma(7)

        # ---- phase 2 (qp-independent part): squares on Act after its copies ----
        sq = p2.tile([128, 3072], F16)
        nc.scalar.square(sq[:], tmp[:])

        # ---- phase 3 + late phase 2, interleaved for engine-order ----
        bankA = psm.tile([128, 512], F32, name="bankA")
        bankB = psm.tile([128, 512], F32, name="bankB")

        def supertile(i):
            npt = npts[i]
            prodAB = prp.tile([128, FS], F16, name=f"prodAB{i}", tag="prodAB")
            prod2 = prp.tile([128, HS], F16, name=f"prod2{i}", tag="prod2")
            # one merged multiply over both c_lo halves (amortizes DVE op cost)
            qwb = (
                qwG[:]
                .rearrange("p (cl u) -> p cl u", cl=2, u=MG)[:, :, i * MSUP:(i + 1) * MSUP]
                .rearrange("p cl (one mm) -> p cl one mm", one=1, mm=MSUP)
                .broadcast_to((128, 2, S, MSUP))
            )
            nc.vector.tensor_mul(
                prodAB[:].rearrange("p (cl s mm) -> p cl s mm", cl=2, s=S, mm=MSUP),
                npt[:].rearrange("p (cl s mm) -> p cl s mm", cl=2, s=S, mm=MSUP),
                qwb,
            )
            add_eng = nc.gpsimd if i in POOL_ADDS else nc.vector
            add_eng.tensor_add(prod2[:], prodAB[:, 0:HS], prodAB[:, HS:FS])

            prod3 = prod2[:].rearrange("p (s mm) -> p s mm", s=S, mm=MSUP)
            for t in range(8):
                d = i * 4 + (t >> 1)
                bank = bankA if (t & 1) == 0 else bankB
                # the group-closing stop rides the last supertile's final
                # matmul per bank (the idn attn2-folds are emitted earlier)
                nc.tensor.matmul(
                    bank[:],
                    lhsT=sel[:, 31 - d:159 - d],
                    rhs=prod3[:, :, t * 32:(t + 1) * 32],
                    start=(i == 0 and t < 2),
                    stop=(i == NSUP - 1 and t >= 6),
                    skip_group_check=True,
                )

        for i in range(3):
            supertile(i)

        # late phase 2 (needs qptB), mostly on Pool - emitted here so neither
        # the DVE supertile stream nor Act's copy stream stalls on the qp
        # scatter.
        n2a = p2.tile([128, 1024], F16)
        ve("n2a").tensor_add(n2a[:], sq[:, 0:1024], sq[:, 1024:2048])
        n2 = p2.tile([128, 1024], F16)
        ve("n2").tensor_add(n2[:], n2a[:], sq[:, 2048:3072])
        norm = p2.tile([128, 1024], F16)
        nc.scalar.sqrt(norm[:], n2[:])

        uw = p2.tile([128, 3072], F16)
        qpb = (
            qptB[:, 0:192]
            .rearrange("p (jb one mm) -> p jb one mm", jb=6, one=1, mm=32)
            .broadcast_to((128, 6, S, 32))
        )
        ve("umul").tensor_mul(
            uw[:].rearrange("p (jb s mm) -> p jb s mm", jb=6, s=S, mm=32), tmp4, qpb
        )
        u1 = p2.tile([128, 1024], F16)
        ve("u1").tensor_add(u1[:], uw[:, 0:1024], uw[:, 1024:2048])
        u = p2.tile([128, 1024], F16)
        ve("u").tensor_add(u[:], u1[:], uw[:, 2048:3072])

        for i in range(3, 6):
            supertile(i)

        # a2/attn2 on DVE, emitted after supertile 5 so the DVE stream reaches
        # them only once `u`/`norm` (Pool/Act) are long since done.
        a2 = p2.tile([128, 1024], F16)
        qp3b = (
            qptB[:, 192:256]
            .rearrange("p (b one mm) -> p b one mm", b=2, one=1, mm=32)
            .broadcast_to((128, 2, S, 32))
        )
        ve("a2").tensor_mul(
            a2[:].rearrange("p (b s mm) -> p b s mm", b=2, s=S, mm=32),
            norm[:].rearrange("p (b s mm) -> p b s mm", b=2, s=S, mm=32),
            qp3b,
        )
        ve("attn2").tensor_add(attn2[:], u[:], a2[:])

        supertile(6)
        # fold the positional term into PSUM mid-group (supertile 7's final
        # matmuls close the groups, keeping the idn adds off the tail)
        nc.tensor.matmul(bankA[:], lhsT=idn[:], rhs=attn2[:, 0:512],
                         start=False, stop=False, skip_group_check=True)
        nc.tensor.matmul(bankB[:], lhsT=idn[:], rhs=attn2[:, 512:1024],
                         start=False, stop=False, skip_group_check=True)
        supertile(7)

        # ---- phase 4: softmax over s (no max subtraction; |logit| < ~3) ----
        for b, bank in ((0, bankA), (1, bankB)):
            e = sp.tile([128, 512], F32, name=f"e{b}", tag="e")
            nc.scalar.activation(e[:], bank[:], AF.Exp)
            se = sp.tile([128, 32], F32, name=f"se{b}", tag="se")
            nc.vector.reduce_sum(se[:], e[:].rearrange("p (s mm) -> p mm s", s=S, mm=32), axis=AX.X)
            rse = sp.tile([128, 32], F32, name=f"rse{b}", tag="rse")
            nc.vector.reciprocal(rse[:], se[:])
            o = sp.tile([128, 512], F32, name=f"o{b}", tag="o")
            rb = rse[:].rearrange("p (one mm) -> p one mm", one=1, mm=32).broadcast_to((128, S, 32))
            nc.vector.tensor_mul(
                o[:].rearrange("p (s mm) -> p s mm", s=S, mm=32),
                e[:].rearrange("p (s mm) -> p s mm", s=S, mm=32),
                rb,
            )
            eng = nc.sync if b == 0 else nc.scalar
            eng.dma_start(OUTD[:, b * 512:(b + 1) * 512], o[:])


_NC_CACHE = None


def build_nc():
    global _NC_CACHE
    if _NC_CACHE is None:
        nc = bacc.Bacc(trn_type="TRN2", target_bir_lowering=False, debug=False)
        with tile.TileContext(nc) as tc:
            _body(tc)
        nc.compile()
        _NC_CACHE = nc
    return _NC_CACHE


def make_in_maps(xyz, neighbor_xyz, points, neighbor_points, Wk, Wpos, bpos):
    """Slice + relayout full inputs into the 8 per-core input maps (fp16)."""
    xyz = np.asarray(xyz, dtype=np.float32)
    neighbor_xyz = np.asarray(neighbor_xyz, dtype=np.float32)
    points = np.asarray(points, dtype=np.float32)
    neighbor_points = np.asarray(neighbor_points, dtype=np.float32)
    Wk = np.asarray(Wk, dtype=np.float32)
    Wpos = np.asarray(Wpos, dtype=np.float32)

    # combined phase-1 lhsT blocks: per h, cols = [qW block | qp cols | zeros]
    WKS = SCALE * Wk                     # [c_in, c_out]; qW = WKS^T @ q
    WQ4 = SCALE * (Wk @ Wpos)            # [c_in, 4];     qp = WQ4^T @ q
    WKH = np.zeros((C, 256), dtype=np.float16)
    WKH[:, 0:64] = WKS                   # h=0: qW rows 0:64
    WKH[:, 64:68] = WQ4                  #      qp rows 64:68
    WKH[:, 128 + 64:128 + 128] = WKS     # h=1: qW rows 64:128
    WKH[:, 128:128 + 4] = WQ4            #      qp rows 0:4
    SEL = np.zeros((128, 160), dtype=np.float16)
    for g in range(4):
        SEL[g * 32:(g + 1) * 32, 31 + g * 32] = 1.0
    IDN = np.eye(128, dtype=np.float16)

    in_maps = []
    for i in range(NCORES):
        nsl = slice(i * NL, (i + 1) * NL)
        # np: [B,C,nl,S] -> [C, M, S] -> [(g, c_hi), (i8, c_lo2, s16, mm256)]
        npc = neighbor_points[:, :, nsl, :].transpose(1, 0, 2, 3).reshape(C, M, S)
        NP = (
            npc.reshape(2, 32, 4, NSUP, MSUP, S)   # [c_lo, c_hi, g, i, mm, s]
            .transpose(2, 1, 3, 0, 5, 4)           # [g, c_hi, i, c_lo, s, mm]
            .reshape(128, NSUP * 2 * S * MSUP)
            .astype(np.float16)
        )
        # nx: [B,3,nl,S] -> [3, M, S] -> [p, (j, b, s, mm)]  (m = p*64+b*32+mm)
        nxc = neighbor_xyz[:, :, nsl, :].transpose(1, 0, 2, 3).reshape(3, M, S)
        NX = (
            nxc.reshape(3, 128, 2, 32, S)
            .transpose(1, 0, 2, 4, 3)
            .reshape(128, 3 * 2 * S * 32)
            .astype(np.float16)
        )
        xc = xyz[:, :, nsl].transpose(1, 0, 2).reshape(3, M)
        XYZ = (
            xc.reshape(3, 128, 2, 32)
            .transpose(1, 0, 2, 3)
            .reshape(128, 3 * 2 * 32)
            .astype(np.float16)
        )
        P = points[:, :, nsl].transpose(1, 0, 2).reshape(C, M).astype(np.float16)
        in_maps.append(
            {
                "NP": np.ascontiguousarray(NP),
                "NX": np.ascontiguousarray(NX),
                "XYZ": np.ascontiguousarray(XYZ),
                "P": np.ascontiguousarray(P),
                "WKH": WKH,
                "SEL": SEL,
                "IDN": IDN,
            }
        )
    return in_maps


def assemble_output(results):
    """Per-core OUT [128, (b2, s16, mm32)] -> full [B, N, S]."""
    out = np.empty((B, N, S), dtype=np.float32)
    for i in range(NCORES):
        oc = np.asarray(results[i]["OUT"]).reshape(128, 2, S, 32)
        full = oc.transpose(0, 1, 3, 2).reshape(M, S)  # m = p*64 + b*32 + mm
        out[:, i * NL:(i + 1) * NL, :] = full.reshape(B, NL, S)
    return out


def run_cores(in_maps, trace=False, trace_kwargs=None):
    nc = build_nc()
    return run_bass_kernel_spmd(
        nc,
        in_maps,
        core_ids=list(range(NCORES)),
        trace=trace,
        **(trace_kwargs or {}),
    )


def kernel(xyz, neighbor_xyz, points, neighbor_points, Wk, Wpos, bpos):
    in_maps = make_in_maps(
        xyz, neighbor_xyz, points, neighbor_points, Wk, Wpos, bpos
    )
    res = run_cores(in_maps, trace=False)
    return assemble_output(res.results)
